# revision 2
# baseline (speedup 1.0000x reference)
"""Trainium2 Bass kernel for nn_LogisticRegressionModel (polynomial-feature logistic regression).

Math: the reference computes sigmoid(poly_features(x) @ W.T + b) where poly_features
are all monomials of x (dim 16) up to degree 4, each degree soft-weighted by
w_d = sigmoid(10*(M - d + 0.5)), M = sigmoid(M_raw)*3 + 1.

Every monomial of degree <= 4 over x embeds as a degree-4 monomial over x1 = [x, 1]
(pad with the constant slot, index 16). Folding W, b, M_raw into a symmetrized
coefficient tensor S4 [289, 289] (built on host, O(P) work), the model becomes
logit_i = (x1 (x) x1)^T S4 (x1 (x) x1). The outer product is symmetric, so it is
further folded onto the 153 unordered pairs of 17 symbols using a mod-17 "wrap"
enumeration p=(d,j) <-> {j, (j+d)%17}, d=0..8: S153 = B^T S4 B. The wrap pairs
have regular strides against a doubled x1 buffer, so one DVE tensor_tensor with
broadcast APs builds XXs[128,153] per batch tile. Then PE transposes XXs
(2 chunks), 2 accumulating matmuls against resident S153 give Y = XXs @ S153,
and a fused scalar_tensor_tensor computes q = rowsum(XXs * Y); sigmoid; store.

Sharding: pure data-parallel over the batch, 4096 rows per core x 8 cores.
"""
import sys
import numpy as np
from itertools import combinations_with_replacement, permutations

sys.path.insert(0, "/opt/trn_rl_repo")

import concourse.bass as bass
import concourse.bacc as bacc
import concourse.tile as tile
from concourse import mybir, masks
from concourse import bass_utils

BATCH = 32768
D = 16
DA = 17            # features + constant slot
ND = 9             # wrap distances 0..8
PD = ND * DA       # 153 unordered pairs
MAX_DEGREE = 4
N_CORES = 8
B_CORE = BATCH // N_CORES   # 4096
N_TILES = B_CORE // 128     # 32
KCH = [128, PD - 128]       # 153 split across partition chunks
P_FULL = 1 + sum(
    len(list(combinations_with_replacement(range(D), d))) for d in range(1, MAX_DEGREE + 1)
)


def _build_s153(W, b, M_raw):
    """Fold W, b and the soft degree weights into the symmetric quartic
    coefficient matrix over the 153 wrap-encoded unordered pairs."""
    W = np.asarray(W, np.float64)
    bval = float(np.asarray(b).reshape(-1)[0])
    M = 1.0 / (1.0 + np.exp(-float(np.asarray(M_raw)))) * (MAX_DEGREE - 1) + 1.0
    coef = {(16, 16, 16, 16): float(W[0, 0]) + bval}
    col = 1
    for d in range(1, MAX_DEGREE + 1):
        w_d = 1.0 / (1.0 + np.exp(-10.0 * (M - d + 0.5)))
        for t in combinations_with_replacement(range(D), d):
            tup = tuple(sorted(t + (16,) * (4 - d)))
            coef[tup] = float(W[0, col]) * w_d
            col += 1
    assert col == P_FULL
    S4 = np.zeros((DA * DA, DA * DA), np.float64)
    for tup, c in coef.items():
        perms = set(permutations(tup))
        v = c / len(perms)
        for (a, b2, c2, d2) in perms:
            S4[a * DA + b2, c2 * DA + d2] += v
    # fold ordered 289-space onto wrap-encoded 153-space
    lookup = {}
    for p, (a, c) in enumerate((j, (j + dd) % DA) for dd in range(ND) for j in range(DA)):
        lookup[(a, c)] = p
        lookup[(c, a)] = p
    B = np.zeros((DA * DA, PD))
    for j in range(DA):
        for k in range(DA):
            B[j * DA + k, lookup[(j, k)]] = 1.0
    return (B.T @ S4 @ B).astype(np.float32)


def _build_nc():
    nc = bacc.Bacc("TRN2", target_bir_lowering=False, debug=False, enable_asserts=False)
    # host pre-packs x1 as [128, N_TILES*34]: partition p, tile t holds x1[t*128+p] twice
    x_d = nc.dram_tensor("x", [128, N_TILES * 2 * DA], mybir.dt.float32, kind="ExternalInput").ap()
    s_d = nc.dram_tensor("s4", [PD, PD], mybir.dt.float32, kind="ExternalInput").ap()
    out_d = nc.dram_tensor("out", [B_CORE, 1], mybir.dt.float32, kind="ExternalOutput").ap()

    f32 = mybir.dt.float32
    with tile.TileContext(nc) as tc:
        with (
            tc.tile_pool(name="const", bufs=1) as const_pool,
            tc.tile_pool(name="xx", bufs=5) as xx_pool,
            tc.tile_pool(name="xxt", bufs=4) as xxt_pool,
            tc.tile_pool(name="prod", bufs=3) as prod_pool,
            tc.tile_pool(name="tr_ps", bufs=5, space="PSUM") as trps_pool,
            tc.tile_pool(name="y_ps", bufs=2, space="PSUM") as yps_pool,
            tc.tile_pool(name="o_ps", bufs=1, space="PSUM") as ops_pool,
        ):
            ident = const_pool.tile([128, 128], f32)
            masks.make_identity(nc, ident[:])
            # S153 chunks: chunk c lives at [:KCH[c], c*153:(c+1)*153]
            s_sb = const_pool.tile([128, 2 * PD], f32)
            for c in range(2):
                nc.sync.dma_start(
                    out=s_sb[: KCH[c], c * PD : (c + 1) * PD],
                    in_=s_d[c * 128 : c * 128 + KCH[c], :],
                )
            qall = const_pool.tile([128, N_TILES], f32)
            oall = const_pool.tile([128, N_TILES], f32)
            xall = const_pool.tile([128, N_TILES * 2 * DA], f32)
            nc.sync.dma_start(out=xall[:], in_=x_d[:])
            # collapse the prologue's many DMA-queue semaphores into one edge
            tc.strict_bb_all_engine_barrier()

            for t in range(N_TILES):
                xc = xall[:, t * 2 * DA : (t + 1) * 2 * DA]

                # XXs[p, d*17+j] = x1[p,j] * x1[p,(j+d)%17] — one DVE op
                xx = xx_pool.tile([128, PD], f32)
                in1 = bass.AP(xc.tensor, xc.offset, [list(xc.ap[0]), [1, ND], [1, DA]])
                nc.vector.tensor_tensor(
                    out=xx[:].rearrange("p (d j) -> p d j", d=ND),
                    in0=xc[:, :DA].unsqueeze(1).broadcast_to([128, ND, DA]),
                    in1=in1,
                    op=mybir.AluOpType.mult,
                )

                # Transpose XXs -> chunks [KCH[c], 128] at cols c*128
                xxt = xxt_pool.tile([128, 2 * 128], f32)
                for c in range(2):
                    trp = trps_pool.tile([128, 128], f32)
                    nc.tensor.transpose(
                        out=trp[: KCH[c], :],
                        in_=xx[:, c * 128 : c * 128 + KCH[c]],
                        identity=ident[:],
                    )
                    nc.scalar.copy(
                        out=xxt[: KCH[c], c * 128 : c * 128 + 128],
                        in_=trp[: KCH[c], :],
                    )

                # Y = XXs @ S153  [128, 153] accumulated over 2 K-chunks
                y_ps = yps_pool.tile([128, PD], f32)
                for c in range(2):
                    nc.tensor.matmul(
                        out=y_ps[:],
                        lhsT=xxt[: KCH[c], c * 128 : c * 128 + 128],
                        rhs=s_sb[: KCH[c], c * PD : (c + 1) * PD],
                        start=(c == 0),
                        stop=(c == 1),
                    )

                # q = rowsum(XXs * Y) — fused multiply + accumulate on DVE
                prod = prod_pool.tile([128, PD], f32)
                nc.vector.scalar_tensor_tensor(
                    out=prod[:],
                    in0=xx[:],
                    scalar=1.0,
                    in1=y_ps[:],
                    op0=mybir.AluOpType.bypass,
                    op1=mybir.AluOpType.mult,
                    accum_out=qall[:, t : t + 1],
                )

            # sigmoid over all 32 tile-columns at once
            nc.scalar.activation(oall[:], qall[:], mybir.ActivationFunctionType.Sigmoid)
            # transpose [128, 32] -> [32, 128] so the DRAM store is contiguous
            o_ps = ops_pool.tile([N_TILES, 128], f32)
            nc.tensor.transpose(out=o_ps[:], in_=oall[:], identity=ident[:])
            o_sb = const_pool.tile([N_TILES, 128], f32)
            nc.vector.tensor_copy(out=o_sb[:], in_=o_ps[:])
            nc.sync.dma_start(
                out=out_d.rearrange("(t p) one -> t (p one)", p=128),
                in_=o_sb[:],
            )
    nc.compile()
    return nc


_NC_CACHE = None


def _pack_x(x):
    x1 = np.concatenate([x, np.ones((x.shape[0], 1), np.float32)], axis=1)
    # pack per core: [N_TILES, 128, 17] -> [128, N_TILES, 2*17] (doubled for wrap reads)
    xr = x1.reshape(N_CORES, N_TILES, 128, DA)
    xp = np.concatenate([xr, xr], axis=3).transpose(0, 2, 1, 3)  # [C, 128, T, 34]
    return np.ascontiguousarray(xp.reshape(N_CORES, 128, N_TILES * 2 * DA))


def kernel(x, W, b, M_raw):
    global _NC_CACHE
    x = np.asarray(x, np.float32)
    xp = _pack_x(x)
    S = _build_s153(W, b, M_raw)
    if _NC_CACHE is None:
        _NC_CACHE = _build_nc()
    nc = _NC_CACHE
    in_maps = [{"x": xp[i], "s4": S} for i in range(N_CORES)]
    res = bass_utils.run_bass_kernel_spmd(nc, in_maps, core_ids=list(range(N_CORES)))
    out = np.concatenate([res.results[i]["out"] for i in range(N_CORES)], axis=0)
    return out.reshape(BATCH, 1).astype(np.float32)


if __name__ == "__main__":
    x = np.random.randn(BATCH, D).astype(np.float32)
    W = (np.random.randn(1, P_FULL) * 0.02).astype(np.float32)
    b = np.zeros((1,), np.float32)
    M_raw = np.zeros((), np.float32)
    out = kernel(x, W, b, M_raw)
    print("out shape:", out.shape, out.dtype, out[:4, 0])



# revision 11
# speedup vs baseline: 1.6133x; 1.6133x over previous
"""Trainium2 Bass kernel for nn_LogisticRegressionModel (polynomial-feature logistic regression).

Math: reference computes sigmoid(poly_features(x) @ W.T + b), poly features = all
monomials of x (dim 16) up to degree 4, soft-weighted per degree. Every monomial
embeds as a degree-4 monomial over x1 = [x, 1] (17 symbols). Folding W, b, M_raw
into a symmetric quartic matrix over the 153 wrap-encoded unordered pairs
p=(d,j) <-> {j,(j+d)%17}: logit_i = XX_i^T S153 XX_i with XX_i[p] = x1_i[a] x1_i[b].

Device pipeline (feature-major / transposed layout, per 512-sample group):
  XX^T[p, s] = X9[p, s] * X9s[p, s]        -- DVE fp16 (2 elem/cy), X9/X9s are
                                              host-replicated row-gathers of x1^T
  Z = U^T XX^T  (S153 = U diag(sign) U^T)  -- 4 fp16 matmuls, stationary U resident
  P = Z^2                                  -- ScalarE Square, PSUM -> SBUF fp16
  q = sign^T P                             -- 2 fp16 matmuls (K=128 + K=25)
  out = sigmoid(q)                         -- ScalarE, batched 4 groups per op
The 25-row tail chunks (153 = 128 + 25) are packed 4 groups deep into 32-partition
PSUM bands via matmul tile_position, so their squares/sigmoid amortize 4x.

Sharding: pure data-parallel over the batch, 4096 rows per core x 8 cores.
"""
import sys
import numpy as np
from itertools import combinations_with_replacement, permutations

sys.path.insert(0, "/opt/trn_rl_repo")

import concourse.bass as bass
import concourse.bacc as bacc
import concourse.tile as tile
from concourse import mybir
from concourse import bass_utils

BATCH = 32768
D = 16
DA = 17            # features + constant slot
ND = 9             # wrap distances 0..8
PD = ND * DA       # 153 unordered pairs
K0, K1 = 128, PD - 128
MAX_DEGREE = 4
N_CORES = 8
B_CORE = BATCH // N_CORES   # 4096
GW = 512                    # group width (PSUM bank = 512 fp32)
N_GROUPS = B_CORE // GW     # 8
P_FULL = 1 + sum(
    len(list(combinations_with_replacement(range(D), d))) for d in range(1, MAX_DEGREE + 1)
)

# wrap pair tables (row p of XX^T multiplies x1 rows PAIR_A[p] * PAIR_B[p])
PAIR_A = np.array([j for d in range(ND) for j in range(DA)], np.int64)
PAIR_B = np.array([(j + d) % DA for d in range(ND) for j in range(DA)], np.int64)


def _build_s153(W, b, M_raw):
    """Fold W, b and the soft degree weights into the symmetric quartic
    coefficient matrix over the 153 wrap-encoded unordered pairs."""
    W = np.asarray(W, np.float64)
    bval = float(np.asarray(b).reshape(-1)[0])
    M = 1.0 / (1.0 + np.exp(-float(np.asarray(M_raw)))) * (MAX_DEGREE - 1) + 1.0
    coef = {(16, 16, 16, 16): float(W[0, 0]) + bval}
    col = 1
    for d in range(1, MAX_DEGREE + 1):
        w_d = 1.0 / (1.0 + np.exp(-10.0 * (M - d + 0.5)))
        for t in combinations_with_replacement(range(D), d):
            tup = tuple(sorted(t + (16,) * (4 - d)))
            coef[tup] = float(W[0, col]) * w_d
            col += 1
    assert col == P_FULL
    S4 = np.zeros((DA * DA, DA * DA), np.float64)
    for tup, c in coef.items():
        perms = set(permutations(tup))
        v = c / len(perms)
        for (a, b2, c2, d2) in perms:
            S4[a * DA + b2, c2 * DA + d2] += v
    lookup = {}
    for p, (a, c) in enumerate(zip(PAIR_A, PAIR_B)):
        lookup[(a, c)] = p
        lookup[(c, a)] = p
    B = np.zeros((DA * DA, PD))
    for j in range(DA):
        for k in range(DA):
            B[j * DA + k, lookup[(j, k)]] = 1.0
    return B.T @ S4 @ B  # float64 [153, 153]


def _build_usign(S):
    """Eigendecompose S153 -> (U fp16 [153,153], sign-banded fp16 [89,1])."""
    lam, V = np.linalg.eigh(S)
    U = (V * np.sqrt(np.abs(lam))[None, :]).astype(np.float16)  # columns scaled
    sign = np.sign(lam).astype(np.float16)
    sg0 = sign[:K0].reshape(K0, 1)
    sg1b = np.zeros((64 + K1, 1), np.float16)  # [89,1], bands at 32*g'
    for gp in range(3):
        sg1b[32 * gp : 32 * gp + K1, 0] = sign[K0:]
    return U, sg0, sg1b


def _build_nc():
    nc = bacc.Bacc("TRN2", target_bir_lowering=False, debug=False, enable_asserts=False)
    f16 = mybir.dt.float16
    f32 = mybir.dt.float32
    # host-prepared replicated pair operands, feature-major fp16
    x9c0_d = nc.dram_tensor("x9c0", [K0, B_CORE], f16, kind="ExternalInput").ap()
    x9c1_d = nc.dram_tensor("x9c1", [K1, B_CORE], f16, kind="ExternalInput").ap()
    x9sc0_d = nc.dram_tensor("x9sc0", [K0, B_CORE], f16, kind="ExternalInput").ap()
    x9sc1_d = nc.dram_tensor("x9sc1", [K1, B_CORE], f16, kind="ExternalInput").ap()
    u0_d = nc.dram_tensor("u0", [K0, PD], f16, kind="ExternalInput").ap()
    u1_d = nc.dram_tensor("u1", [K1, PD], f16, kind="ExternalInput").ap()
    sg0_d = nc.dram_tensor("sg0", [K0, 1], f16, kind="ExternalInput").ap()
    sg1_d = nc.dram_tensor("sg1", [64 + K1, 1], f16, kind="ExternalInput").ap()
    out_d = nc.dram_tensor("out", [N_GROUPS, GW], f32, kind="ExternalOutput").ap()

    with tile.TileContext(nc) as tc:
        with (
            tc.tile_pool(name="const", bufs=1) as const_pool,
            tc.tile_pool(name="x9", bufs=3) as x9_pool,
            tc.tile_pool(name="xx", bufs=3) as xx_pool,
            tc.tile_pool(name="p0", bufs=6) as p0_pool,
            tc.tile_pool(name="p1", bufs=1) as p1_pool,
            tc.tile_pool(name="osb", bufs=1) as o_pool,
            tc.tile_pool(name="z0ps", bufs=2, space="PSUM") as z0_pool,
            tc.tile_pool(name="z1ps", bufs=1, space="PSUM") as z1_pool,
            tc.tile_pool(name="qps", bufs=1, space="PSUM") as q_pool,
        ):
            # resident constants
            u0_sb = const_pool.tile([K0, PD], f16)
            u1_sb = const_pool.tile([K1, PD], f16)
            sg0_sb = const_pool.tile([K0, 1], f16)
            sg1_sb = const_pool.tile([64 + K1, 1], f16)
            nc.sync.dma_start(out=u0_sb[:], in_=u0_d[:])
            nc.sync.dma_start(out=u1_sb[:], in_=u1_d[:])
            nc.sync.dma_start(out=sg0_sb[:], in_=sg0_d[:])
            nc.sync.dma_start(out=sg1_sb[:], in_=sg1_d[:])
            # warm the sigmoid table-set early (Square is a filler in every set)
            warm = const_pool.tile([1, 1], f32)
            nc.vector.memset(warm[:], 0.0)
            nc.scalar.activation(warm[:], warm[:], mybir.ActivationFunctionType.Sigmoid)

            # zero the banded PSUM tiles once so band-gap partitions are defined
            z1_tiles = [z1_pool.tile([64 + K1, GW], f32, name=f"z1t{i}") for i in range(2)]
            q_tiles = [q_pool.tile([65, GW], f32, name=f"qt{i}") for i in range(2)]
            for t in z1_tiles + q_tiles:
                nc.vector.memset(t[:], 0.0)

            p1_tiles = [p1_pool.tile([64 + K1, GW], f16, name=f"p1t{i}") for i in range(2)]
            o_tiles = [o_pool.tile([65, GW], f32, name=f"ot{i}") for i in range(2)]

            p0_window = [None] * 3
            WINDOWS = [list(range(s, min(s + 3, N_GROUPS))) for s in range(0, N_GROUPS, 3)]
            G2W = {g: (wi, g - w[0]) for wi, w in enumerate(WINDOWS) for g in w}
            for g in range(N_GROUPS):
                wi, gp = G2W[g]
                h = wi
                sl = slice(g * GW, (g + 1) * GW)
                z1_ps = z1_tiles[h % 2]
                q_ps = q_tiles[h % 2]
                p1_sb = p1_tiles[h % 2]
                o_sb = o_tiles[h % 2]

                # DMA in this group's slices of the replicated pair operands
                x9c0 = x9_pool.tile([K0, GW], f16, name="x9c0")
                x9c1 = x9_pool.tile([K1, GW], f16, name="x9c1")
                x9sc0 = x9_pool.tile([K0, GW], f16, name="x9sc0")
                x9sc1 = x9_pool.tile([K1, GW], f16, name="x9sc1")
                nc.sync.dma_start(out=x9c0[:], in_=x9c0_d[:, sl])
                nc.sync.dma_start(out=x9c1[:], in_=x9c1_d[:, sl])
                nc.sync.dma_start(out=x9sc0[:], in_=x9sc0_d[:, sl])
                nc.sync.dma_start(out=x9sc1[:], in_=x9sc1_d[:, sl])

                # XX^T = X9 * X9s  (DVE fp16, 2x mode)
                xx0 = xx_pool.tile([K0, GW], f16, name="xx0")
                xx1 = xx_pool.tile([K1, GW], f16, name="xx1")
                nc.vector.tensor_tensor(
                    out=xx0[:], in0=x9c0[:], in1=x9sc0[:], op=mybir.AluOpType.mult)
                nc.vector.tensor_tensor(
                    out=xx1[:], in0=x9c1[:], in1=x9sc1[:], op=mybir.AluOpType.mult)

                # Z rows 0..127 -> own bank; rows 128..152 -> 32-part band of z1_ps
                z0_ps = z0_pool.tile([K0, GW], f32, name="z0_ps")
                nc.tensor.matmul(out=z0_ps[:], lhsT=u0_sb[:, :K0], rhs=xx0[:],
                                 start=True, stop=False)
                nc.tensor.matmul(out=z0_ps[:], lhsT=u1_sb[:, :K0], rhs=xx1[:],
                                 start=False, stop=True)
                band = slice(32 * gp, 32 * gp + K1)
                nc.tensor.matmul(out=z1_ps[band, :], lhsT=u0_sb[:, K0:], rhs=xx0[:],
                                 start=True, stop=False)
                nc.tensor.matmul(out=z1_ps[band, :], lhsT=u1_sb[:, K0:], rhs=xx1[:],
                                 start=False, stop=True)

                # P = Z^2 (ScalarE): chunk0 per group; chunk1 once per 4 groups
                p0_sb = p0_pool.tile([K0, GW], f16, name="p0")
                nc.scalar.activation(p0_sb[:], z0_ps[:],
                                     mybir.ActivationFunctionType.Square)
                p0_window[gp] = p0_sb

                win = WINDOWS[wi]
                if gp == len(win) - 1:
                    nc.scalar.activation(p1_sb[:], z1_ps[:],
                                         mybir.ActivationFunctionType.Square)
                    # q bands = sign^T P (accumulate K chunks per band of q_ps)
                    for gp2 in range(len(win)):
                        band2 = slice(32 * gp2, 32 * gp2 + K1)
                        qb = q_ps[32 * gp2 : 32 * gp2 + 1, :]
                        nc.tensor.matmul(out=qb, lhsT=sg0_sb[:],
                                         rhs=p0_window[gp2][:],
                                         start=True, stop=False)
                        nc.tensor.matmul(out=qb, lhsT=sg1_sb[band2, :],
                                         rhs=p1_sb[band2, :],
                                         start=False, stop=True)
                    # sigmoid + store, rows 0/32/64 of q_ps
                    nc.scalar.activation(o_sb[:], q_ps[:],
                                         mybir.ActivationFunctionType.Sigmoid)
                    for gp2 in range(len(win)):
                        nc.sync.dma_start(
                            out=out_d[win[0] + gp2 : win[0] + gp2 + 1, :],
                            in_=o_sb[32 * gp2 : 32 * gp2 + 1, :])
    nc.compile()
    return nc


_NC_CACHE = None


def _pack_x(x):
    """Per-core replicated fp16 pair operands X9 (rows x1[PAIR_A]) and X9s
    (rows x1[PAIR_B]), each split into 128+25 partition chunks."""
    x1 = np.concatenate([x, np.ones((x.shape[0], 1), np.float32)], axis=1)
    x1t = np.ascontiguousarray(x1.reshape(N_CORES, B_CORE, DA).transpose(0, 2, 1))
    x1t = x1t.astype(np.float16)  # [C, 17, B_CORE]
    X9 = x1t[:, PAIR_A, :]   # [C, 153, B]
    X9s = x1t[:, PAIR_B, :]
    return (np.ascontiguousarray(X9[:, :K0]), np.ascontiguousarray(X9[:, K0:]),
            np.ascontiguousarray(X9s[:, :K0]), np.ascontiguousarray(X9s[:, K0:]))


def _make_in_maps(x, W, b, M_raw):
    x = np.asarray(x, np.float32)
    a0, a1, b0, b1 = _pack_x(x)
    S = _build_s153(W, b, M_raw)
    U, sg0, sg1b = _build_usign(S)
    return [{
        "x9c0": a0[i], "x9c1": a1[i], "x9sc0": b0[i], "x9sc1": b1[i],
        "u0": np.ascontiguousarray(U[:K0]), "u1": np.ascontiguousarray(U[K0:]),
        "sg0": sg0, "sg1": sg1b,
    } for i in range(N_CORES)]


def kernel(x, W, b, M_raw):
    global _NC_CACHE
    in_maps = _make_in_maps(x, W, b, M_raw)
    if _NC_CACHE is None:
        _NC_CACHE = _build_nc()
    nc = _NC_CACHE
    res = bass_utils.run_bass_kernel_spmd(nc, in_maps, core_ids=list(range(N_CORES)))
    out = np.concatenate([res.results[i]["out"].reshape(B_CORE) for i in range(N_CORES)])
    return out.reshape(BATCH, 1).astype(np.float32)


if __name__ == "__main__":
    x = np.random.randn(BATCH, D).astype(np.float32)
    W = (np.random.randn(1, P_FULL) * 0.02).astype(np.float32)
    b = np.zeros((1,), np.float32)
    M_raw = np.zeros((), np.float32)
    out = kernel(x, W, b, M_raw)
    print("out shape:", out.shape, out.dtype, out[:4, 0])


# revision 12
# speedup vs baseline: 1.8780x; 1.1641x over previous
"""Trainium2 Bass kernel for nn_LogisticRegressionModel (polynomial-feature logistic regression).

Math: reference computes sigmoid(poly_features(x) @ W.T + b), poly features = all
monomials of x (dim 16) up to degree 4, soft-weighted per degree. Every monomial
embeds as a degree-4 monomial over x1 = [x, 1] (17 symbols). Folding W, b, M_raw
into a symmetric quartic matrix over the 153 wrap-encoded unordered pairs
p=(d,j) <-> {j,(j+d)%17}: logit_i = XX_i^T S153 XX_i with XX_i[p] = x1_i[a] x1_i[b].

Device pipeline (feature-major layout, per 512-sample group, all fp16 / fp32 PSUM):
  XX^T[p, s] = X9[p, s] * X9s[p, s]        -- DVE, X9/X9s host-replicated x1^T rows
  Z = U^T XX^T  (S153 = U diag(sign) U^T)  -- 4 matmuls, stationary U resident
  P = Z^2                                  -- ScalarE Square, PSUM -> SBUF
  q = sign^T P                             -- 2 matmuls (K=128 + K=25)
  out = sigmoid(q)                         -- ScalarE, batched per window
153 = 128 + 25; the 25-row tail lives in 32-partition PSUM bands (3 groups per
window at bases 0/32/64) so tail squares and sigmoid amortize 3x. Each window's
q-matmuls/sigmoid are deferred into the next window so the PE never stalls on
the ScalarE squares. Inputs ship as 2 packed pair tensors (3 slice DMAs each),
constants as 1 packed tensor, outputs as 3 batched band DMAs -- DMA instruction
count is what the HWDGE pipe charges for.

Sharding: pure data-parallel over the batch, 4096 rows per core x 8 cores.
"""
import sys
import numpy as np
from itertools import combinations_with_replacement, permutations

sys.path.insert(0, "/opt/trn_rl_repo")

import concourse.bass as bass
import concourse.bacc as bacc
import concourse.tile as tile
from concourse import mybir
from concourse import bass_utils

BATCH = 32768
D = 16
DA = 17            # features + constant slot
ND = 9             # wrap distances 0..8
PD = ND * DA       # 153 unordered pairs
K0, K1 = 128, PD - 128
MAX_DEGREE = 4
N_CORES = 8
B_CORE = BATCH // N_CORES   # 4096
GW = 512                    # group width (PSUM bank = 512 fp32)
N_GROUPS = B_CORE // GW     # 8
WINDOWS = [[0, 1, 2], [3, 4, 5], [6, 7]]
NCOL = 310                  # packed const columns: 153 u0 | 153 u1 | sg0 | sg1
P_FULL = 1 + sum(
    len(list(combinations_with_replacement(range(D), d))) for d in range(1, MAX_DEGREE + 1)
)

# wrap pair tables (row p of XX^T multiplies x1 rows PAIR_A[p] * PAIR_B[p])
PAIR_A = np.array([j for d in range(ND) for j in range(DA)], np.int64)
PAIR_B = np.array([(j + d) % DA for d in range(ND) for j in range(DA)], np.int64)


def _build_s153(W, b, M_raw):
    """Fold W, b and the soft degree weights into the symmetric quartic
    coefficient matrix over the 153 wrap-encoded unordered pairs."""
    W = np.asarray(W, np.float64)
    bval = float(np.asarray(b).reshape(-1)[0])
    M = 1.0 / (1.0 + np.exp(-float(np.asarray(M_raw)))) * (MAX_DEGREE - 1) + 1.0
    coef = {(16, 16, 16, 16): float(W[0, 0]) + bval}
    col = 1
    for d in range(1, MAX_DEGREE + 1):
        w_d = 1.0 / (1.0 + np.exp(-10.0 * (M - d + 0.5)))
        for t in combinations_with_replacement(range(D), d):
            tup = tuple(sorted(t + (16,) * (4 - d)))
            coef[tup] = float(W[0, col]) * w_d
            col += 1
    assert col == P_FULL
    S4 = np.zeros((DA * DA, DA * DA), np.float64)
    for tup, c in coef.items():
        perms = set(permutations(tup))
        v = c / len(perms)
        for (a, b2, c2, d2) in perms:
            S4[a * DA + b2, c2 * DA + d2] += v
    lookup = {}
    for p, (a, c) in enumerate(zip(PAIR_A, PAIR_B)):
        lookup[(a, c)] = p
        lookup[(c, a)] = p
    B = np.zeros((DA * DA, PD))
    for j in range(DA):
        for k in range(DA):
            B[j * DA + k, lookup[(j, k)]] = 1.0
    return B.T @ S4 @ B  # float64 [153, 153]


def _build_const(S):
    """Eigendecompose S153 and pack U + sign vectors into one [128, 310] fp16."""
    lam, V = np.linalg.eigh(S)
    U = (V * np.sqrt(np.abs(lam))[None, :]).astype(np.float16)  # columns scaled
    sign = np.sign(lam).astype(np.float16)
    cst = np.zeros((K0, NCOL), np.float16)
    cst[:, :PD] = U[:K0]                    # u0 [128, 153]
    cst[:K1, PD:2 * PD] = U[K0:]            # u1 [25, 153]
    cst[:, 306] = sign[:K0]                 # sg0
    for gp in range(3):                     # sg1 banded at 32*gp
        cst[32 * gp : 32 * gp + K1, 307] = sign[K0:]
    return cst


def _build_nc():
    nc = bacc.Bacc("TRN2", target_bir_lowering=False, debug=False, enable_asserts=False)
    f16 = mybir.dt.float16
    f32 = mybir.dt.float32
    # packed pair operands: [:, 0, :] = X9 rows, [:, 1, :] = X9s rows
    pa_d = nc.dram_tensor("pa", [K0, 2, B_CORE], f16, kind="ExternalInput").ap()
    pb_d = nc.dram_tensor("pb", [K1, 2, B_CORE], f16, kind="ExternalInput").ap()
    cst_d = nc.dram_tensor("cst", [K0, NCOL], f16, kind="ExternalInput").ap()
    out_d = nc.dram_tensor("out", [N_GROUPS, GW], f32, kind="ExternalOutput").ap()

    with tile.TileContext(nc) as tc:
        with (
            tc.tile_pool(name="const", bufs=1) as const_pool,
            tc.tile_pool(name="xx", bufs=4) as xx_pool,
            tc.tile_pool(name="p0", bufs=8) as p0_pool,
            tc.tile_pool(name="z0ps", bufs=3, space="PSUM") as z0_pool,
            tc.tile_pool(name="z1ps", bufs=1, space="PSUM") as z1_pool,
            tc.tile_pool(name="qps", bufs=1, space="PSUM") as q_pool,
        ):
            # resident constants + staged inputs
            cst = const_pool.tile([K0, NCOL], f16)
            nc.sync.dma_start(out=cst[:], in_=cst_d[:])
            u0 = cst[:, 0:PD]
            u1 = cst[:K1, PD:2 * PD]
            sg0 = cst[:, 306:307]
            sg1 = cst[:, 307:308]

            pa = const_pool.tile([K0, 2, B_CORE], f16)
            pb = const_pool.tile([K1, 2, B_CORE], f16)
            # group-0 slice first, then the remainder in two big slices
            for lo, hi in ((0, GW), (GW, 4 * GW), (4 * GW, 8 * GW)):
                nc.sync.dma_start(out=pa[:, :, lo:hi], in_=pa_d[:, :, lo:hi])
                nc.sync.dma_start(out=pb[:, :, lo:hi], in_=pb_d[:, :, lo:hi])

            # warm the sigmoid table-set early (Square co-resides in every set)
            warm = const_pool.tile([1, 1], f32)
            nc.vector.memset(warm[:], 0.0)
            nc.scalar.activation(warm[:], warm[:], mybir.ActivationFunctionType.Sigmoid)

            # banded PSUM tiles (bands at 0/32/64); zero once so gaps are defined
            z1_tiles = [z1_pool.tile([64 + K1, GW], f32, name=f"z1t{i}") for i in range(2)]
            q_tiles = [q_pool.tile([65, GW], f32, name=f"qt{i}") for i in range(2)]
            for t in z1_tiles + q_tiles:
                nc.vector.memset(t[:], 0.0)

            p1_tiles = [const_pool.tile([64 + K1, GW], f16, name=f"p1t{i}") for i in range(2)]
            o_all = const_pool.tile([65, len(WINDOWS) * GW], f32)

            p0_win = {}

            def finish_window(wi):
                """Emit window wi's tail: p1 square, q-matmuls, sigmoid."""
                win = WINDOWS[wi]
                z1_ps = z1_tiles[wi % 2]
                q_ps = q_tiles[wi % 2]
                p1_sb = p1_tiles[wi % 2]
                nc.scalar.activation(p1_sb[:], z1_ps[:],
                                     mybir.ActivationFunctionType.Square)
                for gp in range(len(win)):
                    band = slice(32 * gp, 32 * gp + K1)
                    qb = q_ps[32 * gp : 32 * gp + 1, :]
                    nc.tensor.matmul(out=qb, lhsT=sg0, rhs=p0_win.pop(win[gp])[:],
                                     start=True, stop=False)
                    nc.tensor.matmul(out=qb, lhsT=sg1[band, :], rhs=p1_sb[band, :],
                                     start=False, stop=True)
                nc.scalar.activation(o_all[:, wi * GW:(wi + 1) * GW], q_ps[:],
                                     mybir.ActivationFunctionType.Sigmoid)

            for wi, win in enumerate(WINDOWS):
                for gp, g in enumerate(win):
                    sl = slice(g * GW, (g + 1) * GW)
                    z1_ps = z1_tiles[wi % 2]

                    # XX^T = X9 * X9s  (DVE fp16 2x mode)
                    xx0 = xx_pool.tile([K0, GW], f16, name="xx0")
                    xx1 = xx_pool.tile([K1, GW], f16, name="xx1")
                    nc.vector.tensor_tensor(
                        out=xx0[:], in0=pa[:, 0, sl], in1=pa[:, 1, sl],
                        op=mybir.AluOpType.mult)
                    nc.vector.tensor_tensor(
                        out=xx1[:], in0=pb[:, 0, sl], in1=pb[:, 1, sl],
                        op=mybir.AluOpType.mult)

                    # Z rows 0..127 -> own bank; rows 128..152 -> band of z1_ps
                    z0_ps = z0_pool.tile([K0, GW], f32, name="z0_ps")
                    nc.tensor.matmul(out=z0_ps[:], lhsT=u0[:, :K0], rhs=xx0[:],
                                     start=True, stop=False)
                    nc.tensor.matmul(out=z0_ps[:], lhsT=u1[:, :K0], rhs=xx1[:],
                                     start=False, stop=True)
                    band = slice(32 * gp, 32 * gp + K1)
                    nc.tensor.matmul(out=z1_ps[band, :], lhsT=u0[:, K0:], rhs=xx0[:],
                                     start=True, stop=False)
                    nc.tensor.matmul(out=z1_ps[band, :], lhsT=u1[:, K0:], rhs=xx1[:],
                                     start=False, stop=True)

                    # defer the previous window's tail until after this window's
                    # first z-matmuls so its squares never stall the PE
                    if gp == 0 and wi > 0:
                        finish_window(wi - 1)

                    # P0 = Z0^2 (ScalarE, PSUM -> SBUF fp16)
                    p0_sb = p0_pool.tile([K0, GW], f16, name="p0")
                    nc.scalar.activation(p0_sb[:], z0_ps[:],
                                         mybir.ActivationFunctionType.Square)
                    p0_win[g] = p0_sb

            finish_window(len(WINDOWS) - 1)

            # batched output: band b of o_all -> dram rows {3w + b}
            for b2 in range(3):
                nw = sum(1 for w in WINDOWS if len(w) > b2)
                dst = bass.AP(out_d.tensor, b2 * GW, [[3 * GW, nw], [1, GW]])
                nc.sync.dma_start(out=dst, in_=o_all[32 * b2 : 32 * b2 + 1, : nw * GW])
    nc.compile()
    return nc


_NC_CACHE = None


def _pack_x(x):
    """Per-core packed fp16 pair operands: pa [128, 2, B] rows 0..127 of
    (X9|X9s), pb [25, 2, B] rows 128..152."""
    x1 = np.concatenate([x, np.ones((x.shape[0], 1), np.float32)], axis=1)
    x1t = np.ascontiguousarray(x1.reshape(N_CORES, B_CORE, DA).transpose(0, 2, 1))
    x1t = x1t.astype(np.float16)  # [C, 17, B_CORE]
    X9 = x1t[:, PAIR_A, :]   # [C, 153, B]
    X9s = x1t[:, PAIR_B, :]
    pa = np.ascontiguousarray(np.stack([X9[:, :K0], X9s[:, :K0]], axis=2))
    pb = np.ascontiguousarray(np.stack([X9[:, K0:], X9s[:, K0:]], axis=2))
    return pa, pb  # [C,128,2,B], [C,25,2,B]


def _make_in_maps(x, W, b, M_raw):
    x = np.asarray(x, np.float32)
    pa, pb = _pack_x(x)
    cst = _build_const(_build_s153(W, b, M_raw))
    return [{"pa": pa[i], "pb": pb[i], "cst": cst} for i in range(N_CORES)]


def kernel(x, W, b, M_raw):
    global _NC_CACHE
    in_maps = _make_in_maps(x, W, b, M_raw)
    if _NC_CACHE is None:
        _NC_CACHE = _build_nc()
    nc = _NC_CACHE
    res = bass_utils.run_bass_kernel_spmd(nc, in_maps, core_ids=list(range(N_CORES)))
    out = np.concatenate([res.results[i]["out"].reshape(B_CORE) for i in range(N_CORES)])
    return out.reshape(BATCH, 1).astype(np.float32)


if __name__ == "__main__":
    x = np.random.randn(BATCH, D).astype(np.float32)
    W = (np.random.randn(1, P_FULL) * 0.02).astype(np.float32)
    b = np.zeros((1,), np.float32)
    M_raw = np.zeros((), np.float32)
    out = kernel(x, W, b, M_raw)
    print("out shape:", out.shape, out.dtype, out[:4, 0])


# revision 13
# speedup vs baseline: 1.9171x; 1.0209x over previous
"""Trainium2 Bass kernel for nn_LogisticRegressionModel (polynomial-feature logistic regression).

Math: reference computes sigmoid(poly_features(x) @ W.T + b), poly features = all
monomials of x (dim 16) up to degree 4, soft-weighted per degree. Every monomial
embeds as a degree-4 monomial over x1 = [x, 1] (17 symbols). Folding W, b, M_raw
into a symmetric quartic matrix over the 153 wrap-encoded unordered pairs
p=(d,j) <-> {j,(j+d)%17}: logit_i = XX_i^T S153 XX_i with XX_i[p] = x1_i[a] x1_i[b].

Device pipeline (feature-major layout, per 512-sample group, all fp16 / fp32 PSUM):
  XX^T[p, s] = X9[p, s] * X9s[p, s]        -- DVE, X9/X9s host-replicated x1^T rows
  Z = U^T XX^T  (S153 = U diag(sign) U^T)  -- 4 matmuls, stationary U resident
  P = Z^2                                  -- ScalarE Square, PSUM -> SBUF
  q = sign^T P                             -- 2 matmuls (K=128 + K=25)
  out = sigmoid(q)                         -- ScalarE, batched per window
153 = 128 + 25; the 25-row tail lives in 32-partition PSUM bands (3 groups per
window at bases 0/32/64) so tail squares and sigmoid amortize 3x. Each window's
q-matmuls/sigmoid are deferred into the next window so the PE never stalls on
the ScalarE squares. Inputs ship as 2 packed pair tensors (3 slice DMAs each),
constants as 1 packed tensor, outputs as 3 batched band DMAs -- DMA instruction
count is what the HWDGE pipe charges for.

Sharding: pure data-parallel over the batch, 4096 rows per core x 8 cores.
"""
import sys
import numpy as np
from itertools import combinations_with_replacement, permutations

sys.path.insert(0, "/opt/trn_rl_repo")

import concourse.bass as bass
import concourse.bacc as bacc
import concourse.tile as tile
from concourse import mybir
from concourse import bass_utils

BATCH = 32768
D = 16
DA = 17            # features + constant slot
ND = 9             # wrap distances 0..8
PD = ND * DA       # 153 unordered pairs
K0, K1 = 128, PD - 128
MAX_DEGREE = 4
N_CORES = 8
B_CORE = BATCH // N_CORES   # 4096
GW = 512                    # group width (PSUM bank = 512 fp32)
N_GROUPS = B_CORE // GW     # 8
WINDOWS = [[0, 1, 2], [3, 4, 5], [6, 7]]
NCOL = 310                  # packed const columns: 153 u0 | 153 u1 | sg0 | sg1
P_FULL = 1 + sum(
    len(list(combinations_with_replacement(range(D), d))) for d in range(1, MAX_DEGREE + 1)
)

# wrap pair tables (row p of XX^T multiplies x1 rows PAIR_A[p] * PAIR_B[p])
PAIR_A = np.array([j for d in range(ND) for j in range(DA)], np.int64)
PAIR_B = np.array([(j + d) % DA for d in range(ND) for j in range(DA)], np.int64)


def _build_s153(W, b, M_raw):
    """Fold W, b and the soft degree weights into the symmetric quartic
    coefficient matrix over the 153 wrap-encoded unordered pairs."""
    W = np.asarray(W, np.float64)
    bval = float(np.asarray(b).reshape(-1)[0])
    M = 1.0 / (1.0 + np.exp(-float(np.asarray(M_raw)))) * (MAX_DEGREE - 1) + 1.0
    coef = {(16, 16, 16, 16): float(W[0, 0]) + bval}
    col = 1
    for d in range(1, MAX_DEGREE + 1):
        w_d = 1.0 / (1.0 + np.exp(-10.0 * (M - d + 0.5)))
        for t in combinations_with_replacement(range(D), d):
            tup = tuple(sorted(t + (16,) * (4 - d)))
            coef[tup] = float(W[0, col]) * w_d
            col += 1
    assert col == P_FULL
    S4 = np.zeros((DA * DA, DA * DA), np.float64)
    for tup, c in coef.items():
        perms = set(permutations(tup))
        v = c / len(perms)
        for (a, b2, c2, d2) in perms:
            S4[a * DA + b2, c2 * DA + d2] += v
    lookup = {}
    for p, (a, c) in enumerate(zip(PAIR_A, PAIR_B)):
        lookup[(a, c)] = p
        lookup[(c, a)] = p
    B = np.zeros((DA * DA, PD))
    for j in range(DA):
        for k in range(DA):
            B[j * DA + k, lookup[(j, k)]] = 1.0
    return B.T @ S4 @ B  # float64 [153, 153]


def _build_const(S):
    """Eigendecompose S153 and pack U + sign vectors into one [128, 310] fp16."""
    lam, V = np.linalg.eigh(S)
    U = (V * np.sqrt(np.abs(lam))[None, :]).astype(np.float16)  # columns scaled
    sign = np.sign(lam).astype(np.float16)
    cst = np.zeros((K0, NCOL), np.float16)
    cst[:, :PD] = U[:K0]                    # u0 [128, 153]
    cst[:K1, PD:2 * PD] = U[K0:]            # u1 [25, 153]
    cst[:, 306] = sign[:K0]                 # sg0
    for gp in range(3):                     # sg1 banded at 32*gp
        cst[32 * gp : 32 * gp + K1, 307] = sign[K0:]
    return cst


def _build_nc():
    nc = bacc.Bacc("TRN2", target_bir_lowering=False, debug=False, enable_asserts=False)
    f16 = mybir.dt.float16
    f32 = mybir.dt.float32
    # packed pair operands: [:, 0, :] = X9 rows, [:, 1, :] = X9s rows
    pa_d = nc.dram_tensor("pa", [K0, 2, B_CORE], f16, kind="ExternalInput").ap()
    pb_d = nc.dram_tensor("pb", [K1, 2, B_CORE], f16, kind="ExternalInput").ap()
    cst_d = nc.dram_tensor("cst", [K0, NCOL], f16, kind="ExternalInput").ap()
    out_d = nc.dram_tensor("out", [N_GROUPS, GW], f32, kind="ExternalOutput").ap()

    with tile.TileContext(nc) as tc:
        with (
            tc.tile_pool(name="const", bufs=1) as const_pool,
            tc.tile_pool(name="xx", bufs=12) as xx_pool,
            tc.tile_pool(name="p0", bufs=8) as p0_pool,
            tc.tile_pool(name="z0ps", bufs=4, space="PSUM") as z0_pool,
            tc.tile_pool(name="z1ps", bufs=1, space="PSUM") as z1_pool,
            tc.tile_pool(name="qps", bufs=1, space="PSUM") as q_pool,
        ):
            # resident constants + staged inputs
            cst = const_pool.tile([K0, NCOL], f16)
            u0 = cst[:, 0:PD]
            u1 = cst[:K1, PD:2 * PD]
            sg0 = cst[:, 306:307]
            sg1 = cst[:, 307:308]

            pa = const_pool.tile([K0, 2, B_CORE], f16)
            pb = const_pool.tile([K1, 2, B_CORE], f16)
            # progressive slices sized so slice k lands before DVE consumes it;
            # cst rides after the first pair (needed by the first matmul)
            for si, (lo, hi) in enumerate(((0, GW), (GW, 3 * GW), (3 * GW, 5 * GW),
                                           (5 * GW, 8 * GW))):
                nc.sync.dma_start(out=pa[:, :, lo:hi], in_=pa_d[:, :, lo:hi])
                nc.sync.dma_start(out=pb[:, :, lo:hi], in_=pb_d[:, :, lo:hi])
                if si == 0:
                    nc.sync.dma_start(out=cst[:], in_=cst_d[:])

            # warm the sigmoid table-set early (Square co-resides in every set)
            warm = const_pool.tile([1, 1], f32)
            nc.vector.memset(warm[:], 0.0)
            nc.scalar.activation(warm[:], warm[:], mybir.ActivationFunctionType.Sigmoid)

            # banded PSUM tiles (bands at 0/32/64); zero once so gaps are defined
            z1_tiles = [z1_pool.tile([64 + K1, GW], f32, name=f"z1t{i}") for i in range(2)]
            q_tiles = [q_pool.tile([65, GW], f32, name=f"qt{i}") for i in range(2)]
            for t in z1_tiles + q_tiles:
                nc.vector.memset(t[:], 0.0)

            p1_tiles = [const_pool.tile([64 + K1, GW], f16, name=f"p1t{i}") for i in range(2)]
            o_all = const_pool.tile([65, len(WINDOWS) * GW], f32)

            p0_win = {}

            def finish_window(wi):
                """Emit window wi's tail: p1 square, q-matmuls, sigmoid."""
                win = WINDOWS[wi]
                z1_ps = z1_tiles[wi % 2]
                q_ps = q_tiles[wi % 2]
                p1_sb = p1_tiles[wi % 2]
                nc.scalar.activation(p1_sb[:], z1_ps[:],
                                     mybir.ActivationFunctionType.Square)
                for gp in range(len(win)):
                    band = slice(32 * gp, 32 * gp + K1)
                    qb = q_ps[32 * gp : 32 * gp + 1, :]
                    nc.tensor.matmul(out=qb, lhsT=sg0, rhs=p0_win.pop(win[gp])[:],
                                     start=True, stop=False)
                    nc.tensor.matmul(out=qb, lhsT=sg1[band, :], rhs=p1_sb[band, :],
                                     start=False, stop=True)
                nc.scalar.activation(o_all[:, wi * GW:(wi + 1) * GW], q_ps[:],
                                     mybir.ActivationFunctionType.Sigmoid)

            for wi, win in enumerate(WINDOWS):
                for gp, g in enumerate(win):
                    sl = slice(g * GW, (g + 1) * GW)
                    z1_ps = z1_tiles[wi % 2]

                    # XX^T = X9 * X9s  (DVE fp16 2x mode)
                    xx0 = xx_pool.tile([K0, GW], f16, name="xx0")
                    xx1 = xx_pool.tile([K1, GW], f16, name="xx1")
                    nc.vector.tensor_tensor(
                        out=xx0[:], in0=pa[:, 0, sl], in1=pa[:, 1, sl],
                        op=mybir.AluOpType.mult)
                    nc.vector.tensor_tensor(
                        out=xx1[:], in0=pb[:, 0, sl], in1=pb[:, 1, sl],
                        op=mybir.AluOpType.mult)

                    # Z rows 0..127 -> own bank; rows 128..152 -> band of z1_ps
                    z0_ps = z0_pool.tile([K0, GW], f32, name="z0_ps")
                    nc.tensor.matmul(out=z0_ps[:], lhsT=u0[:, :K0], rhs=xx0[:],
                                     start=True, stop=False)
                    nc.tensor.matmul(out=z0_ps[:], lhsT=u1[:, :K0], rhs=xx1[:],
                                     start=False, stop=True)
                    band = slice(32 * gp, 32 * gp + K1)
                    nc.tensor.matmul(out=z1_ps[band, :], lhsT=u0[:, K0:], rhs=xx0[:],
                                     start=True, stop=False)
                    nc.tensor.matmul(out=z1_ps[band, :], lhsT=u1[:, K0:], rhs=xx1[:],
                                     start=False, stop=True)

                    # defer the previous window's tail until after this window's
                    # first z-matmuls so its squares never stall the PE
                    if gp == 0 and wi > 0:
                        finish_window(wi - 1)

                    # P0 = Z0^2 (ScalarE, PSUM -> SBUF fp16)
                    p0_sb = p0_pool.tile([K0, GW], f16, name="p0")
                    nc.scalar.activation(p0_sb[:], z0_ps[:],
                                         mybir.ActivationFunctionType.Square)
                    p0_win[g] = p0_sb

            finish_window(len(WINDOWS) - 1)

            # batched output: band b of o_all -> dram rows {3w + b}
            for b2 in range(3):
                nw = sum(1 for w in WINDOWS if len(w) > b2)
                dst = bass.AP(out_d.tensor, b2 * GW, [[3 * GW, nw], [1, GW]])
                nc.sync.dma_start(out=dst, in_=o_all[32 * b2 : 32 * b2 + 1, : nw * GW])
    nc.compile()
    return nc


_NC_CACHE = None


def _pack_x(x):
    """Per-core packed fp16 pair operands: pa [128, 2, B] rows 0..127 of
    (X9|X9s), pb [25, 2, B] rows 128..152."""
    x1 = np.concatenate([x, np.ones((x.shape[0], 1), np.float32)], axis=1)
    x1t = np.ascontiguousarray(x1.reshape(N_CORES, B_CORE, DA).transpose(0, 2, 1))
    x1t = x1t.astype(np.float16)  # [C, 17, B_CORE]
    X9 = x1t[:, PAIR_A, :]   # [C, 153, B]
    X9s = x1t[:, PAIR_B, :]
    pa = np.ascontiguousarray(np.stack([X9[:, :K0], X9s[:, :K0]], axis=2))
    pb = np.ascontiguousarray(np.stack([X9[:, K0:], X9s[:, K0:]], axis=2))
    return pa, pb  # [C,128,2,B], [C,25,2,B]


def _make_in_maps(x, W, b, M_raw):
    x = np.asarray(x, np.float32)
    pa, pb = _pack_x(x)
    cst = _build_const(_build_s153(W, b, M_raw))
    return [{"pa": pa[i], "pb": pb[i], "cst": cst} for i in range(N_CORES)]


def kernel(x, W, b, M_raw):
    global _NC_CACHE
    in_maps = _make_in_maps(x, W, b, M_raw)
    if _NC_CACHE is None:
        _NC_CACHE = _build_nc()
    nc = _NC_CACHE
    res = bass_utils.run_bass_kernel_spmd(nc, in_maps, core_ids=list(range(N_CORES)))
    out = np.concatenate([res.results[i]["out"].reshape(B_CORE) for i in range(N_CORES)])
    return out.reshape(BATCH, 1).astype(np.float32)


if __name__ == "__main__":
    x = np.random.randn(BATCH, D).astype(np.float32)
    W = (np.random.randn(1, P_FULL) * 0.02).astype(np.float32)
    b = np.zeros((1,), np.float32)
    M_raw = np.zeros((), np.float32)
    out = kernel(x, W, b, M_raw)
    print("out shape:", out.shape, out.dtype, out[:4, 0])


# revision 21
# speedup vs baseline: 2.3240x; 1.2122x over previous
"""Trainium2 Bass kernel for nn_LogisticRegressionModel (polynomial-feature logistic regression).

Math: reference computes sigmoid(poly_features(x) @ W.T + b), poly features = all
monomials of x (dim 16) up to degree 4, soft-weighted per degree. Every monomial
embeds as a degree-4 monomial over x1 = [x, 1] (17 symbols). Folding W, b, M_raw
into a symmetric quartic matrix over the 153 wrap-encoded unordered pairs
p=(d,j) <-> {j,(j+d)%17}: logit_i = XX_i^T S153 XX_i with XX_i[p] = x1_i[a] x1_i[b].

Device pipeline (feature-major layout, per 512-sample group, all fp16 / fp32 PSUM):
  XX^T[p, s] = X9[p, s] * X9s[p, s]        -- DVE, X9/X9s host-replicated x1^T rows
  Z = U^T XX^T  (S153 = U diag(sign) U^T)  -- 4 matmuls, stationary U resident
  P = Z^2                                  -- ScalarE Square, PSUM -> SBUF
  q = sign^T P                             -- 2 matmuls (K=128 + K=25)
  out = sigmoid(q)                         -- ScalarE, batched per window
153 = 128 + 25; the 25-row tail lives in 32-partition PSUM bands (3 groups per
window at bases 0/32/64) so tail squares and sigmoid amortize 3x. Each window's
q-matmuls/sigmoid are deferred into the next window so the PE never stalls on
the ScalarE squares. Inputs ship as 2 packed pair tensors (3 slice DMAs each),
constants as 1 packed tensor, outputs as 3 batched band DMAs -- DMA instruction
count is what the HWDGE pipe charges for.

Sharding: pure data-parallel over the batch, 4096 rows per core x 8 cores.
"""
import sys
import numpy as np
from itertools import combinations_with_replacement, permutations

sys.path.insert(0, "/opt/trn_rl_repo")

import concourse.bass as bass
import concourse.bacc as bacc
import concourse.tile as tile
from concourse import mybir
from concourse import bass_utils

BATCH = 32768
D = 16
DA = 17            # features + constant slot
ND = 9             # wrap distances 0..8
PD = ND * DA       # 153 unordered pairs
K0, K1 = 128, PD - 128
MAX_DEGREE = 4
N_CORES = 8
B_CORE = BATCH // N_CORES   # 4096
GW = 512                    # group width (PSUM bank = 512 fp32)
N_GROUPS = B_CORE // GW     # 8
WINDOWS = [[0, 1, 2], [3, 4, 5], [6, 7]]
NCOL = 310                  # packed const columns: 153 u0 | 153 u1 | sg0 | sg1
P_FULL = 1 + sum(
    len(list(combinations_with_replacement(range(D), d))) for d in range(1, MAX_DEGREE + 1)
)

# wrap pair tables (row p of XX^T multiplies x1 rows PAIR_A[p] * PAIR_B[p])
PAIR_A = np.array([j for d in range(ND) for j in range(DA)], np.int64)
PAIR_B = np.array([(j + d) % DA for d in range(ND) for j in range(DA)], np.int64)


def _build_s153(W, b, M_raw):
    """Fold W, b and the soft degree weights into the symmetric quartic
    coefficient matrix over the 153 wrap-encoded unordered pairs."""
    W = np.asarray(W, np.float64)
    bval = float(np.asarray(b).reshape(-1)[0])
    M = 1.0 / (1.0 + np.exp(-float(np.asarray(M_raw)))) * (MAX_DEGREE - 1) + 1.0
    coef = {(16, 16, 16, 16): float(W[0, 0]) + bval}
    col = 1
    for d in range(1, MAX_DEGREE + 1):
        w_d = 1.0 / (1.0 + np.exp(-10.0 * (M - d + 0.5)))
        for t in combinations_with_replacement(range(D), d):
            tup = tuple(sorted(t + (16,) * (4 - d)))
            coef[tup] = float(W[0, col]) * w_d
            col += 1
    assert col == P_FULL
    S4 = np.zeros((DA * DA, DA * DA), np.float64)
    for tup, c in coef.items():
        perms = set(permutations(tup))
        v = c / len(perms)
        for (a, b2, c2, d2) in perms:
            S4[a * DA + b2, c2 * DA + d2] += v
    lookup = {}
    for p, (a, c) in enumerate(zip(PAIR_A, PAIR_B)):
        lookup[(a, c)] = p
        lookup[(c, a)] = p
    B = np.zeros((DA * DA, PD))
    for j in range(DA):
        for k in range(DA):
            B[j * DA + k, lookup[(j, k)]] = 1.0
    return B.T @ S4 @ B  # float64 [153, 153]


def _build_const(S):
    """Eigendecompose S153 and pack U + sign vectors into one [128, 310] fp16."""
    lam, V = np.linalg.eigh(S)
    U = (V * np.sqrt(np.abs(lam))[None, :]).astype(np.float16)  # columns scaled
    sign = np.sign(lam).astype(np.float16)
    cst = np.zeros((K0, NCOL), np.float16)
    cst[:, :PD] = U[:K0]                    # u0 [128, 153]
    cst[:K1, PD:2 * PD] = U[K0:]            # u1 [25, 153]
    cst[:, 306] = sign[:K0]                 # sg0
    for gp in range(3):                     # sg1 banded at 32*gp
        cst[32 * gp : 32 * gp + K1, 307] = sign[K0:]
    return cst


def _build_nc():
    nc = bacc.Bacc("TRN2", target_bir_lowering=False, debug=False, enable_asserts=False)
    f16 = mybir.dt.float16
    f32 = mybir.dt.float32
    # packed pair operands: [:, 0, :] = X9 rows, [:, 1, :] = X9s rows
    pa_d = nc.dram_tensor("pa", [K0, 2, B_CORE], f16, kind="ExternalInput").ap()
    pb_d = nc.dram_tensor("pb", [K1, 2, B_CORE], f16, kind="ExternalInput").ap()
    cst_d = nc.dram_tensor("cst", [K0, NCOL], f16, kind="ExternalInput").ap()
    out_d = nc.dram_tensor("out", [N_GROUPS, GW], f32, kind="ExternalOutput").ap()

    with tile.TileContext(nc) as tc:
        with (
            tc.tile_pool(name="const", bufs=1) as const_pool,
            tc.tile_pool(name="xx", bufs=12) as xx_pool,
            tc.tile_pool(name="p0", bufs=8) as p0_pool,
            tc.tile_pool(name="z0ps", bufs=4, space="PSUM") as z0_pool,
            tc.tile_pool(name="z1ps", bufs=1, space="PSUM") as z1_pool,
            tc.tile_pool(name="qps", bufs=1, space="PSUM") as q_pool,
        ):
            # resident constants + staged inputs
            cst = const_pool.tile([K0, NCOL], f16)
            u0 = cst[:, 0:PD]
            u1 = cst[:K1, PD:2 * PD]
            sg0 = cst[:, 306:307]
            sg1 = cst[:, 307:308]

            pa = const_pool.tile([K0, 2, B_CORE], f16)
            pb = const_pool.tile([K1, 2, B_CORE], f16)
            # progressive slices sized so slice k lands before DVE consumes it;
            # cst rides after the first pair (needed by the first matmul)
            for si, (lo, hi) in enumerate(((0, GW), (GW, 3 * GW), (3 * GW, 5 * GW),
                                           (5 * GW, 8 * GW))):
                nc.sync.dma_start(out=pa[:, :, lo:hi], in_=pa_d[:, :, lo:hi])
                nc.sync.dma_start(out=pb[:, :, lo:hi], in_=pb_d[:, :, lo:hi])
                if si == 0:
                    nc.sync.dma_start(out=cst[:], in_=cst_d[:])

            # warm the sigmoid table-set early (Square co-resides in every set)
            warm = const_pool.tile([1, 1], f32)
            nc.vector.memset(warm[:], 0.0)
            nc.scalar.activation(warm[:], warm[:], mybir.ActivationFunctionType.Sigmoid)

            # banded PSUM tiles (bands at 0/32/64); zero once so gaps are defined
            z1_tiles = [z1_pool.tile([64 + K1, GW], f32, name=f"z1t{i}") for i in range(2)]
            q_tiles = [q_pool.tile([65, GW], f32, name=f"qt{i}") for i in range(2)]
            for t in z1_tiles + q_tiles:
                nc.vector.memset(t[:], 0.0)

            p1_tiles = [const_pool.tile([64 + K1, GW], f16, name=f"p1t{i}") for i in range(2)]
            o_all = const_pool.tile([65, len(WINDOWS) * GW], f32)

            p0_win = {}

            def window_mms(wi, prev_wi):
                """Window wi's matmuls in same-tile-size blocks (each tile-size
                switch costs ~95ns of PE pipeline restart), with the previous
                window's q-matmuls folded into the matching blocks."""
                win = WINDOWS[wi]
                z1_ps = z1_tiles[wi % 2]
                z0s, xxs = [], []
                for gp, g in enumerate(win):
                    sl = slice(g * GW, (g + 1) * GW)
                    xx0 = xx_pool.tile([K0, GW], f16, name="xx0")
                    xx1 = xx_pool.tile([K1, GW], f16, name="xx1")
                    nc.vector.tensor_tensor(
                        out=xx0[:], in0=pa[:, 0, sl], in1=pa[:, 1, sl],
                        op=mybir.AluOpType.mult)
                    nc.vector.tensor_tensor(
                        out=xx1[:], in0=pb[:, 0, sl], in1=pb[:, 1, sl],
                        op=mybir.AluOpType.mult)
                    xxs.append((xx0, xx1))
                    z0s.append(z0_pool.tile([K0, GW], f32, name="z0_ps"))
                if prev_wi is not None:
                    pwin = WINDOWS[prev_wi]
                    pq_ps = q_tiles[prev_wi % 2]
                    pp1 = p1_tiles[prev_wi % 2]
                # block 1: z0 K0-parts, tile (128,128)
                for gp, g in enumerate(win):
                    nc.tensor.matmul(out=z0s[gp][:], lhsT=u0[:, :K0],
                                     rhs=xxs[gp][0][:], start=True, stop=False,
                                     skip_group_check=True)
                # block 2: z0 K1-parts, tile (32,128) -- closes z0 groups
                for gp, g in enumerate(win):
                    nc.tensor.matmul(out=z0s[gp][:], lhsT=u1[:, :K0],
                                     rhs=xxs[gp][1][:], start=False, stop=True,
                                     skip_group_check=True)
                # block 3: z1 K1-parts, tile (32,32) -- opens z1 bands
                for gp, g in enumerate(win):
                    band = slice(32 * gp, 32 * gp + K1)
                    nc.tensor.matmul(out=z1_ps[band, :], lhsT=u1[:, K0:],
                                     rhs=xxs[gp][1][:], start=True, stop=False,
                                     skip_group_check=True)
                # block 4: prev window's q K1-parts, tile (32,32)
                if prev_wi is not None:
                    for gp in range(len(pwin)):
                        band = slice(32 * gp, 32 * gp + K1)
                        nc.tensor.matmul(out=pq_ps[32 * gp : 32 * gp + 1, :],
                                         lhsT=sg1[band, :], rhs=pp1[band, :],
                                         start=True, stop=False,
                                         skip_group_check=True)
                # block 5: z1 K0-parts, tile (128,32) -- closes z1 bands
                for gp, g in enumerate(win):
                    band = slice(32 * gp, 32 * gp + K1)
                    nc.tensor.matmul(out=z1_ps[band, :], lhsT=u0[:, K0:],
                                     rhs=xxs[gp][0][:], start=False, stop=True,
                                     skip_group_check=True)
                # block 6: prev window's q K0-parts, tile (128,32) -- closes q
                if prev_wi is not None:
                    for gp in range(len(pwin)):
                        nc.tensor.matmul(out=pq_ps[32 * gp : 32 * gp + 1, :],
                                         lhsT=sg0, rhs=p0_win.pop(pwin[gp])[:],
                                         start=False, stop=True,
                                         skip_group_check=True)
                # ScalarE: square z1 first in late windows (drain-phase q needs
                # it), then this window's z0 chunks
                late = wi == len(WINDOWS) - 1
                if late:
                    nc.scalar.activation(p1_tiles[wi % 2][:], z1_ps[:],
                                         mybir.ActivationFunctionType.Square)
                for gp, g in enumerate(win):
                    p0_sb = p0_pool.tile([K0, GW], f16, name="p0")
                    nc.scalar.activation(p0_sb[:], z0s[gp][:],
                                         mybir.ActivationFunctionType.Square)
                    p0_win[g] = p0_sb
                if not late:
                    nc.scalar.activation(p1_tiles[wi % 2][:], z1_ps[:],
                                         mybir.ActivationFunctionType.Square)
                # prev window's sigmoid after its q bands close, then store
                if prev_wi is not None:
                    nc.scalar.activation(
                        o_all[:, prev_wi * GW:(prev_wi + 1) * GW], pq_ps[:],
                        mybir.ActivationFunctionType.Sigmoid)
                    for gp2 in range(len(pwin)):
                        nc.sync.dma_start(
                            out=out_d[pwin[gp2] : pwin[gp2] + 1, :],
                            in_=o_all[32 * gp2 : 32 * gp2 + 1,
                                      prev_wi * GW:(prev_wi + 1) * GW])

            def finish_last(wi):
                """Final window's q-matmuls + sigmoid (no next window to ride)."""
                win = WINDOWS[wi]
                q_ps = q_tiles[wi % 2]
                p1_sb = p1_tiles[wi % 2]
                for gp in range(len(win)):
                    band = slice(32 * gp, 32 * gp + K1)
                    nc.tensor.matmul(out=q_ps[32 * gp : 32 * gp + 1, :],
                                     lhsT=sg1[band, :], rhs=p1_sb[band, :],
                                     start=True, stop=False, skip_group_check=True)
                for gp in range(len(win)):
                    nc.tensor.matmul(out=q_ps[32 * gp : 32 * gp + 1, :],
                                     lhsT=sg0, rhs=p0_win.pop(win[gp])[:],
                                     start=False, stop=True, skip_group_check=True)
                nc.scalar.activation(o_all[:, wi * GW:(wi + 1) * GW], q_ps[:],
                                     mybir.ActivationFunctionType.Sigmoid)
                for gp2 in range(len(win)):
                    nc.sync.dma_start(
                        out=out_d[win[gp2] : win[gp2] + 1, :],
                        in_=o_all[32 * gp2 : 32 * gp2 + 1,
                                  wi * GW:(wi + 1) * GW])

            for wi in range(len(WINDOWS)):
                window_mms(wi, wi - 1 if wi > 0 else None)
            finish_last(len(WINDOWS) - 1)

    nc.compile()
    return nc


_NC_CACHE = None


def _pack_x(x):
    """Per-core packed fp16 pair operands: pa [128, 2, B] rows 0..127 of
    (X9|X9s), pb [25, 2, B] rows 128..152."""
    x1 = np.concatenate([x, np.ones((x.shape[0], 1), np.float32)], axis=1)
    x1t = np.ascontiguousarray(x1.reshape(N_CORES, B_CORE, DA).transpose(0, 2, 1))
    x1t = x1t.astype(np.float16)  # [C, 17, B_CORE]
    X9 = x1t[:, PAIR_A, :]   # [C, 153, B]
    X9s = x1t[:, PAIR_B, :]
    pa = np.ascontiguousarray(np.stack([X9[:, :K0], X9s[:, :K0]], axis=2))
    pb = np.ascontiguousarray(np.stack([X9[:, K0:], X9s[:, K0:]], axis=2))
    return pa, pb  # [C,128,2,B], [C,25,2,B]


def _make_in_maps(x, W, b, M_raw):
    x = np.asarray(x, np.float32)
    pa, pb = _pack_x(x)
    cst = _build_const(_build_s153(W, b, M_raw))
    return [{"pa": pa[i], "pb": pb[i], "cst": cst} for i in range(N_CORES)]


def kernel(x, W, b, M_raw):
    global _NC_CACHE
    in_maps = _make_in_maps(x, W, b, M_raw)
    if _NC_CACHE is None:
        _NC_CACHE = _build_nc()
    nc = _NC_CACHE
    res = bass_utils.run_bass_kernel_spmd(nc, in_maps, core_ids=list(range(N_CORES)))
    out = np.concatenate([res.results[i]["out"].reshape(B_CORE) for i in range(N_CORES)])
    return out.reshape(BATCH, 1).astype(np.float32)


if __name__ == "__main__":
    x = np.random.randn(BATCH, D).astype(np.float32)
    W = (np.random.randn(1, P_FULL) * 0.02).astype(np.float32)
    b = np.zeros((1,), np.float32)
    M_raw = np.zeros((), np.float32)
    out = kernel(x, W, b, M_raw)
    print("out shape:", out.shape, out.dtype, out[:4, 0])
